# revision 1
# baseline (speedup 1.0000x reference)
"""CrossAttentionTransformerBlock on 8 TRN2 NeuronCores (Bass/Tile).

Sharding: core 2b+s handles (batch b, stream s) where s=0 -> x, s=1 -> src.
Self-attn + MLP are stream-local; the bidirectional cross-attention exchanges
(keys, values) between the pair (2b, 2b+1) via pairwise AllReduces
(partner = sum - mine), which keeps the program SPMD-uniform.

Layout: the residual stream lives feature-major xT [C=768, T=1024] in SBUF.
All LN scale/bias are folded into the following projection weights/biases on
the host, so on-chip LN is a pure normalize. Matmuls run in float32r
(full PE rate); softmax probabilities and V are bf16. Softmax has no
max-subtraction (logits are small by construction); denominators come from a
ones-augmented column of V in the AV matmul.
"""

import numpy as np

import concourse.bacc as bacc
import concourse.bass as bass
import concourse.mybir as mybir
import concourse.tile as tile
from concourse.bass_utils import run_bass_kernel_spmd

F32 = mybir.dt.float32
F32R = mybir.dt.float32r
BF16 = mybir.dt.bfloat16
AF = mybir.ActivationFunctionType
OP = mybir.AluOpType

B, T, C, H, HD = 4, 1024, 768, 12, 64
HID = 4 * C
EPS = 1e-6
SCALE = HD ** -0.5
NT = T // 128      # 8 token tiles
KC = C // 128      # 6 feature chunks
NH = T // 512      # 2 moving halves (f32 moving max 512)
NHT = HID // 128   # 24 hidden tiles
N_CORES = 8
GROUPS = [[0, 1], [2, 3], [4, 5], [6, 7]]

_CACHE = {}


def _emit(nc):
    dp = nc.declare_dram_parameter
    tok_d = dp("tok", [T, C], F32R, isOutput=False)
    ident_d = dp("ident", [128, 128], F32R, isOutput=False)
    ones_d = dp("ones", [128, 128], F32R, isOutput=False)
    wqk_d = dp("wqk", [C, 2 * C], F32R, isOutput=False)
    bqk_d = dp("bqk", [2 * C], F32, isOutput=False)
    wvs_d = dp("wvs", [C, C], F32R, isOutput=False)
    wproj_d = dp("wproj", [C, C], F32R, isOutput=False)
    bproj_d = dp("bproj", [C], F32, isOutput=False)
    wab_d = dp("wab", [C, 2 * C], F32R, isOutput=False)
    bab_d = dp("bab", [2 * C], F32, isOutput=False)
    wvc_d = dp("wvc", [C, C], F32R, isOutput=False)
    wcp_d = dp("wcp", [C, C], F32R, isOutput=False)
    bcp_d = dp("bcp", [C], F32, isOutput=False)
    wm1_d = dp("wm1", [C, HID], F32R, isOutput=False)
    bm1_d = dp("bm1", [HID], F32, isOutput=False)
    wm2_d = dp("wm2", [HID, C], F32R, isOutput=False)
    bm2_d = dp("bm2", [C], F32, isOutput=False)
    out_d = dp("out_tok", [T, C], F32, isOutput=True)

    with tile.TileContext(nc) as tc:
        with (
            tc.tile_pool(name="pP", bufs=1) as pP,
            tc.tile_pool(name="pW", bufs=8) as pW,
            tc.tile_pool(name="pB", bufs=4) as pB,
            tc.tile_pool(name="pROW", bufs=4) as pROW,
            tc.tile_pool(name="pSQ", bufs=2) as pSQ,
            tc.tile_pool(name="pPT", bufs=4) as pPT,
            tc.tile_pool(name="pRS", bufs=2) as pRS,
            tc.tile_pool(name="ps", bufs=3, space="PSUM") as psp,
            tc.tile_pool(name="pst", bufs=2, space="PSUM") as pstp,
            tc.tile_pool(name="dram", bufs=1, space="DRAM") as dram,
        ):
            ident = pP.tile([128, 128], F32R, tag="ident")
            onesm = pP.tile([128, 128], F32R, tag="onesm")
            nc.sync.dma_start(out=ident[:], in_=ident_d[:])
            nc.sync.dma_start(out=onesm[:], in_=ones_d[:])
            eps_t = pP.tile([1, 1], F32, tag="epst")
            nc.vector.memset(eps_t[:], EPS)

            def ps_tile(dt=F32):
                return psp.tile([128, 1024], dt, tag="PS", name="pst_")

            def bias_tile(b_dram, off):
                bt = pB.tile([128, 1], F32, tag="BIAS", name="bt")
                nc.sync.dma_start(
                    out=bt[:],
                    in_=b_dram[off:off + 128].rearrange("(p o) -> p o", o=1),
                )
                return bt

            # ---- residual stream tiles (persistent) ----
            xT = [pP.tile([128, T], F32R, tag=f"XT{i}", name=f"xT{i}")
                  for i in range(KC)]

            # ---- phase 0: load tokens, transpose to feature-major ----
            with tc.tile_pool(name="pTOK", bufs=8) as pTOK:
                tok_sb = []
                for tt in range(NT):
                    t_ = pTOK.tile([128, C], F32R, tag="TOK", name="tok_t")
                    nc.sync.dma_start(out=t_[:],
                                      in_=tok_d[tt * 128:(tt + 1) * 128, :])
                    tok_sb.append(t_)
                for cc in range(KC):
                    for tt in range(NT):
                        pt = pstp.tile([128, 512], F32R, tag="TP", name="pt")
                        nc.tensor.transpose(
                            pt[:, 0:128], tok_sb[tt][:, cc * 128:(cc + 1) * 128],
                            ident[:],
                        )
                        nc.vector.tensor_copy(
                            xT[cc][:, tt * 128:(tt + 1) * 128], pt[:, 0:128]
                        )

            # ---- helpers ----
            def layer_norm(src, pXH):
                ps_mu = ps_tile()
                ps_sq = ps_tile()
                for kc in range(KC):
                    sqt = pSQ.tile([128, T], F32R, tag="SQT", name="sqt")
                    nc.vector.tensor_tensor(
                        sqt[:], src[kc][:].bitcast(F32),
                        src[kc][:].bitcast(F32), OP.mult,
                    )
                    for hf in range(NH):
                        sl = slice(hf * 512, (hf + 1) * 512)
                        nc.tensor.matmul(
                            ps_mu[0:1, sl], onesm[:, 0:1], src[kc][:, sl],
                            start=(kc == 0), stop=(kc == KC - 1),
                        )
                        nc.tensor.matmul(
                            ps_sq[0:1, sl], onesm[:, 0:1], sqt[:, sl],
                            start=(kc == 0), stop=(kc == KC - 1),
                        )
                mu = pROW.tile([1, T], F32R, tag="ROW", name="mu")
                with nc.allow_low_precision(reason="f32r row"):
                    nc.vector.tensor_scalar(
                        mu[:], ps_mu[0:1, :], 1.0 / C, None, op0=OP.mult,
                    )
                mv = pROW.tile([1, T], F32, tag="ROW", name="mv")
                nc.vector.tensor_tensor(
                    mv[:], mu[:].bitcast(F32), mu[:].bitcast(F32), OP.mult
                )
                nc.vector.scalar_tensor_tensor(
                    out=mv[:], in0=ps_sq[0:1, :], scalar=1.0 / C, in1=mv[:],
                    op0=OP.mult, op1=OP.subtract,
                )
                nc.scalar.activation(mv[:], mv[:], AF.Ln, bias=eps_t[:])
                rstd = pROW.tile([1, T], F32R, tag="ROW", name="rstd")
                nc.scalar.activation(rstd[:], mv[:], AF.Exp, scale=-0.5)
                mr = pROW.tile([1, T], F32R, tag="ROW", name="mr")
                with nc.allow_low_precision(reason="f32r row"):
                    nc.vector.tensor_tensor(
                        mr[:], mu[:].bitcast(F32), rstd[:].bitcast(F32),
                        OP.mult,
                    )
                ps_rb = ps_tile()
                ps_mb = ps_tile()
                for hf in range(NH):
                    sl = slice(hf * 512, (hf + 1) * 512)
                    nc.tensor.matmul(ps_rb[:, sl], onesm[0:1, :],
                                     rstd[0:1, sl], start=True, stop=True)
                    nc.tensor.matmul(ps_mb[:, sl], onesm[0:1, :],
                                     mr[0:1, sl], start=True, stop=True)
                out = []
                for kc in range(KC):
                    xh = pXH.tile([128, T], F32R, tag="XH", name="xh")
                    nc.vector.tensor_tensor(
                        xh[:], src[kc][:].bitcast(F32), ps_rb[:], OP.mult
                    )
                    nc.vector.tensor_tensor(
                        xh[:], xh[:].bitcast(F32), ps_mb[:], OP.subtract
                    )
                    out.append(xh)
                return out

            def proj_fm(src, w_dram, b_dram, col_off, pool, out_tag):
                outs = []
                for ot in range(KC):
                    wts = []
                    for kc in range(KC):
                        wt = pW.tile([128, 128], F32R, tag="W", name="wt")
                        nc.sync.dma_start(
                            out=wt[:],
                            in_=w_dram[kc * 128:(kc + 1) * 128,
                                       col_off + ot * 128:col_off + (ot + 1) * 128],
                        )
                        wts.append(wt)
                    pp = ps_tile()
                    for hf in range(NH):
                        sl = slice(hf * 512, (hf + 1) * 512)
                        for kc in range(KC):
                            nc.tensor.matmul(
                                pp[:, sl], wts[kc][:], src[kc][:, sl],
                                start=(kc == 0), stop=(kc == KC - 1),
                            )
                    ot_sb = pool.tile([128, T], F32R, tag=out_tag, name="otile")
                    bt = bias_tile(b_dram, col_off + ot * 128)
                    nc.scalar.activation(ot_sb[:], pp[:], AF.Identity,
                                         bias=bt[:], scale=1.0)
                    outs.append(ot_sb)
                return outs

            def proj_vaug(src, w_dram, pWM, pVA):
                wmov = []
                for kc in range(KC):
                    wm = pWM.tile([128, C], F32R, tag="WM", name="wm")
                    nc.sync.dma_start(out=wm[:],
                                      in_=w_dram[kc * 128:(kc + 1) * 128, :])
                    wmov.append(wm)
                outs = []
                for mt in range(NT):
                    pp = ps_tile()
                    for ck in range(2):  # 768 = 512 + 256 moving chunks
                        sl = slice(ck * 512, min((ck + 1) * 512, C))
                        for kc in range(KC):
                            nc.tensor.matmul(
                                pp[:, sl],
                                src[kc][:, mt * 128:(mt + 1) * 128],
                                wmov[kc][:, sl],
                                start=(kc == 0), stop=(kc == KC - 1),
                            )
                    va = pVA.tile([128, 780], BF16, tag="VA", name="va")
                    va3 = va[:].rearrange("p (h e) -> p h e", e=65)
                    nc.scalar.copy(
                        va3[:, :, 0:64],
                        pp[:, 0:C].rearrange("p (h e) -> p h e", e=64),
                    )
                    nc.vector.memset(va3[:, :, 64:65], 1.0)
                    outs.append(va)
                return outs

            def attention(qT, kT, va, pOT):
                oT = [pOT.tile([128, T], F32R, tag="OT", name=f"oT{_i}")
                      for _i in range(KC)]
                for h in range(H):
                    ct, ro = divmod(h * HD, 128)
                    psO = ps_tile()
                    for mt in range(NT):
                        psS = ps_tile()
                        for hf in range(NH):
                            sl = slice(hf * 512, (hf + 1) * 512)
                            nc.tensor.matmul(
                                psS[:, sl],
                                kT[ct][ro:ro + HD, mt * 128:(mt + 1) * 128],
                                qT[ct][ro:ro + HD, sl],
                                start=True, stop=True,
                            )
                        pT = pPT.tile([128, T], BF16, tag="PT", name="pT")
                        nc.scalar.activation(pT[:], psS[:], AF.Exp)
                        for hf in range(NH):
                            sl = slice(hf * 512, (hf + 1) * 512)
                            nc.tensor.matmul(
                                psO[0:65, sl],
                                va[mt][:, h * 65:(h + 1) * 65],
                                pT[:, sl],
                                start=(mt == 0), stop=(mt == NT - 1),
                            )
                    if ro == 0:
                        nc.scalar.copy(oT[ct][0:64, :], psO[0:64, :])
                    else:
                        nc.vector.tensor_copy(oT[ct][64:128, :], psO[0:64, :])
                    # denominator: recip straight off psum row 64
                    rcp = pROW.tile([1, T], F32R, tag="ROW", name="rcp")
                    with nc.allow_low_precision(reason="f32r recip"):
                        nc.vector.reciprocal(rcp[:], psO[64:65, :])
                    for hf in range(NH):
                        sl = slice(hf * 512, (hf + 1) * 512)
                        psR = pstp.tile([128, 512], F32, tag="TP", name="psR")
                        nc.tensor.matmul(
                            psR[0:64, :], onesm[0:1, 0:64],
                            rcp[0:1, sl], start=True, stop=True,
                        )
                        nc.vector.tensor_tensor(
                            oT[ct][ro:ro + 64, sl],
                            oT[ct][ro:ro + 64, sl].bitcast(F32),
                            psR[0:64, :], OP.mult,
                        )
                return oT

            def proj_residual(src, w_dram, b_dram):
                for ot in range(KC):
                    wts = []
                    for kc in range(KC):
                        wt = pW.tile([128, 128], F32R, tag="W", name="wt")
                        nc.sync.dma_start(
                            out=wt[:],
                            in_=w_dram[kc * 128:(kc + 1) * 128,
                                       ot * 128:(ot + 1) * 128],
                        )
                        wts.append(wt)
                    pp = ps_tile()
                    for hf in range(NH):
                        sl = slice(hf * 512, (hf + 1) * 512)
                        for kc in range(KC):
                            nc.tensor.matmul(
                                pp[:, sl], wts[kc][:], src[kc][:, sl],
                                start=(kc == 0), stop=(kc == KC - 1),
                            )
                    bt = bias_tile(b_dram, ot * 128)
                    nc.vector.scalar_tensor_tensor(
                        out=xT[ot][:], in0=pp[:], scalar=bt[:],
                        in1=xT[ot][:].bitcast(F32), op0=OP.add, op1=OP.add,
                    )

            # ================= self-attention =================
            with (
                tc.tile_pool(name="pQK_s", bufs=6) as pQK,
                tc.tile_pool(name="pVA_s", bufs=9) as pVA,
            ):
                with (
                    tc.tile_pool(name="pXH_s", bufs=6) as pXH,
                    tc.tile_pool(name="pWM_s", bufs=6) as pWM,
                ):
                    xh1 = layer_norm(xT, pXH)
                    qT = proj_fm(xh1, wqk_d, bqk_d, 0, pQK, "QT")
                    kT = proj_fm(xh1, wqk_d, bqk_d, C, pQK, "KT")
                    va_s = proj_vaug(xh1, wvs_d, pWM, pVA)
                with tc.tile_pool(name="pOT_s", bufs=6) as pOT:
                    oT = attention(qT, kT, va_s, pOT)
                    proj_residual(oT, wproj_d, bproj_d)

            # ================= cross-attention ================
            with (
                tc.tile_pool(name="pQK_c", bufs=7) as pQK,
                tc.tile_pool(name="pVA_c", bufs=9) as pVA,
            ):
                with (
                    tc.tile_pool(name="pXH_c", bufs=6) as pXH,
                    tc.tile_pool(name="pWM_c", bufs=6) as pWM,
                ):
                    xhc = layer_norm(xT, pXH)
                    bT = proj_fm(xhc, wab_d, bab_d, C, pQK, "KT")
                    va_c = proj_vaug(xhc, wvc_d, pWM, pVA)
                    aT = None

                ccb_in = dram.tile([C, T], F32, tag="ccb_in")
                ccb_out = dram.tile([C, T], F32, tag="ccb_out")
                ccv_in = dram.tile([T, 780], BF16, tag="ccv_in")
                ccv_out = dram.tile([T, 780], BF16, tag="ccv_out")
                for cc in range(KC):
                    nc.sync.dma_start(out=ccb_in[cc * 128:(cc + 1) * 128, :],
                                      in_=bT[cc][:].bitcast(F32))
                for mt in range(NT):
                    nc.sync.dma_start(out=ccv_in[mt * 128:(mt + 1) * 128, :],
                                      in_=va_c[mt][:])
                nc.gpsimd.collective_compute(
                    "AllReduce", OP.add, replica_groups=GROUPS,
                    ins=[ccb_in.opt()], outs=[ccb_out.opt()],
                )
                nc.gpsimd.collective_compute(
                    "AllReduce", OP.add, replica_groups=GROUPS,
                    ins=[ccv_in.opt()], outs=[ccv_out.opt()],
                )
                with (
                    tc.tile_pool(name="pXH_c2", bufs=6) as pXH2,
                ):
                    xhc2 = layer_norm(xT, pXH2)
                    aT = proj_fm(xhc2, wab_d, bab_d, 0, pQK, "QT")
                bP, vP = [], []
                for cc in range(KC):
                    st = pRS.tile([128, T], F32, tag="RS", name="st")
                    nc.sync.dma_start(out=st[:],
                                      in_=ccb_out[cc * 128:(cc + 1) * 128, :])
                    bp = pQK.tile([128, T], F32R, tag="KT", name="bp")
                    nc.vector.tensor_sub(bp[:], st[:], bT[cc][:].bitcast(F32))
                    bP.append(bp)
                for mt in range(NT):
                    sv = pRS.tile([128, 780], BF16, tag="RSV", name="sv")
                    nc.sync.dma_start(out=sv[:],
                                      in_=ccv_out[mt * 128:(mt + 1) * 128, :])
                    vp = pVA.tile([128, 780], BF16, tag="VA", name="vp")
                    nc.vector.tensor_sub(vp[:], sv[:], va_c[mt][:])
                    vP.append(vp)

                with tc.tile_pool(name="pOT_c", bufs=6) as pOT:
                    oTc = attention(aT, bP, vP, pOT)
                    proj_residual(oTc, wcp_d, bcp_d)

            # ================= MLP =================
            with (
                tc.tile_pool(name="pXH_m", bufs=6) as pXH,
                tc.tile_pool(name="pHT", bufs=24) as pHT,
                tc.tile_pool(name="pW2", bufs=26) as pW2,
            ):
                xh2 = layer_norm(xT, pXH)
                for hf in range(NH):
                    sl = slice(hf * 512, (hf + 1) * 512)
                    hT = []
                    for ht in range(NHT):
                        wts = []
                        for kc in range(KC):
                            wt = pW.tile([128, 128], F32R, tag="W", name="wt")
                            nc.sync.dma_start(
                                out=wt[:],
                                in_=wm1_d[kc * 128:(kc + 1) * 128,
                                          ht * 128:(ht + 1) * 128],
                            )
                            wts.append(wt)
                        pp = ps_tile()
                        for kc in range(KC):
                            nc.tensor.matmul(
                                pp[:, 0:512], wts[kc][:], xh2[kc][:, sl],
                                start=(kc == 0), stop=(kc == KC - 1),
                            )
                        bt = bias_tile(bm1_d, ht * 128)
                        h_sb = pHT.tile([128, 512], F32R, tag="HT",
                                        name="h_sb")
                        nc.scalar.activation(h_sb[:], pp[:, 0:512], AF.Gelu,
                                             bias=bt[:], scale=1.0)
                        hT.append(h_sb)
                    for ot in range(KC):
                        wts = []
                        for ht in range(NHT):
                            wt = pW2.tile([128, 128], F32R, tag="W2",
                                          name="wt")
                            nc.sync.dma_start(
                                out=wt[:],
                                in_=wm2_d[ht * 128:(ht + 1) * 128,
                                          ot * 128:(ot + 1) * 128],
                            )
                            wts.append(wt)
                        pp = ps_tile()
                        for ht in range(NHT):
                            nc.tensor.matmul(
                                pp[:, 0:512], wts[ht][:], hT[ht][:],
                                start=(ht == 0), stop=(ht == NHT - 1),
                            )
                        bt = bias_tile(bm2_d, ot * 128)
                        nc.vector.scalar_tensor_tensor(
                            out=xT[ot][:, sl], in0=pp[:, 0:512], scalar=bt[:],
                            in1=xT[ot][:, sl].bitcast(F32),
                            op0=OP.add, op1=OP.add,
                        )

            # ---- output transpose + DMA ----
            with tc.tile_pool(name="pOUT", bufs=2) as pOUT:
                for tt in range(NT):
                    ot_sb = pOUT.tile([128, C], F32, tag="OUTT", name="ot_sb")
                    for cc in range(KC):
                        pt = pstp.tile([128, 512], F32R, tag="TP", name="pt")
                        nc.tensor.transpose(
                            pt[:, 0:128], xT[cc][:, tt * 128:(tt + 1) * 128],
                            ident[:],
                        )
                        nc.vector.tensor_copy(
                            ot_sb[:, cc * 128:(cc + 1) * 128],
                            pt[:, 0:128].bitcast(F32),
                        )
                    nc.sync.dma_start(out=out_d[tt * 128:(tt + 1) * 128, :],
                                      in_=ot_sb[:])

    nc.compile()
    return nc


def _build():
    if "nc" not in _CACHE:
        nc = bacc.Bacc("TRN2", target_bir_lowering=False)
        _CACHE["nc"] = _emit(nc)
    return _CACHE["nc"]


def _fold_ln(w, ln_w, ln_b):
    """w [out, in]; returns (w', b') with LN scale/bias folded in."""
    w = np.asarray(w, np.float64)
    wf = w * np.asarray(ln_w, np.float64)[None, :]
    bf = w @ np.asarray(ln_b, np.float64)
    return wf, bf


def _core_inputs(s, tok, p):
    sfx = "" if s == 0 else "s"
    wqkv, bqkv = _fold_ln(p["w_qkv" + ("" if s == 0 else "_s")],
                          p[f"ln1{sfx}_w"], p[f"ln1{sfx}_b"])
    wqkv = wqkv.copy()
    wqkv[:C] *= SCALE
    bqkv = bqkv.copy()
    bqkv[:C] *= SCALE
    wproj = np.asarray(p["w_proj" + ("" if s == 0 else "_s")], np.float64)
    bproj = np.asarray(p["b_proj" + ("" if s == 0 else "_s")], np.float64) \
        + wproj @ bqkv[2 * C:]
    lncw = p["lnc_w" if s == 0 else "lncs_w"]
    lncb = p["lnc_b" if s == 0 else "lncs_b"]
    wqk, bqk_ = _fold_ln(p["w_qk" if s == 0 else "w_qk_src"], lncw, lncb)
    wqk3 = wqk.reshape(H, 2 * HD, C)
    bqk3 = bqk_.reshape(H, 2 * HD)
    if s == 0:
        A, Ab = wqk3[:, :HD] * SCALE, bqk3[:, :HD] * SCALE
        Bm, Bb = wqk3[:, HD:] * SCALE, bqk3[:, HD:] * SCALE
    else:
        A, Ab = wqk3[:, HD:], bqk3[:, HD:]
        Bm, Bb = wqk3[:, :HD], bqk3[:, :HD]
    wab = np.concatenate([A.reshape(C, C), Bm.reshape(C, C)], axis=0)
    bab = np.concatenate([Ab.reshape(C), Bb.reshape(C)], axis=0)
    wvc, bvc = _fold_ln(p["w_v" if s == 0 else "w_v_src"], lncw, lncb)
    wcp = np.asarray(p["w_cp" if s == 0 else "w_cp_src"], np.float64)
    bcp = np.asarray(p["b_cp" if s == 0 else "b_cp_src"], np.float64) + wcp @ bvc
    wm1, bm1 = _fold_ln(p[f"mlp1{sfx}_w"], p[f"ln2{sfx}_w"], p[f"ln2{sfx}_b"])
    bm1 = bm1 + np.asarray(p[f"mlp1{sfx}_b"], np.float64)
    wm2 = np.asarray(p[f"mlp2{sfx}_w"], np.float64)
    bm2 = np.asarray(p[f"mlp2{sfx}_b"], np.float64)

    f32 = lambda a: np.ascontiguousarray(a, np.float32)
    return {
        "tok": f32(tok),
        "ident": f32(np.eye(128)),
        "ones": f32(np.ones((128, 128))),
        "wqk": f32(wqkv[:2 * C].T),
        "bqk": f32(bqkv[:2 * C]),
        "wvs": f32(wqkv[2 * C:].T),
        "wproj": f32(wproj.T),
        "bproj": f32(bproj),
        "wab": f32(wab.T),
        "bab": f32(bab),
        "wvc": f32(wvc.T),
        "wcp": f32(wcp.T),
        "bcp": f32(bcp),
        "wm1": f32(wm1.T),
        "bm1": f32(bm1),
        "wm2": f32(wm2.T),
        "bm2": f32(bm2),
    }


def make_in_maps(inputs):
    x = np.asarray(inputs["x"])
    src = np.asarray(inputs["src"])
    maps = []
    for b in range(B):
        for s in range(2):
            maps.append(_core_inputs(s, x[b] if s == 0 else src[b], inputs))
    return maps


def kernel(**inputs):
    nc = _build()
    in_maps = make_in_maps(inputs)
    res = run_bass_kernel_spmd(nc, in_maps, list(range(N_CORES)))
    x_out = np.stack([res.results[2 * b]["out_tok"] for b in range(B)])
    src_out = np.stack([res.results[2 * b + 1]["out_tok"] for b in range(B)])
    return (x_out.astype(np.float32), src_out.astype(np.float32))



# revision 11
# speedup vs baseline: 1.9500x; 1.9500x over previous
"""CrossAttentionTransformerBlock on 8 TRN2 NeuronCores (Bass/Tile).

Sharding: core 2b+s handles (batch b, stream s) where s=0 -> x, s=1 -> src.
Self-attn + MLP are stream-local; the bidirectional cross-attention exchanges
(keys, values) between the pair (2b, 2b+1) via pairwise AllReduces
(partner = sum - mine), which keeps the program SPMD-uniform.

v2: all matmuls in bf16 (f32r@N=512 measured ~2x slower on HW; bf16 gets
FWL weight loads), weight DMAs as [128, full-width] slabs, QK^T row-packed
2 heads/issue via tile_position, batched softmax denominators with one
reciprocal_approx_fast per attention, single-load MLP weights, bf16
collectives overlapped with the partner-independent query projection.
Residual stream stays f32 feature-major; PSUM stays f32.
"""

import numpy as np
import ml_dtypes

import concourse.bacc as bacc
import concourse.bass as bass
import concourse.mybir as mybir
import concourse.tile as tile
from concourse.bass_utils import run_bass_kernel_spmd

F32 = mybir.dt.float32
F32R = mybir.dt.float32r
BF16 = mybir.dt.bfloat16
AF = mybir.ActivationFunctionType
OP = mybir.AluOpType

B, T, C, H, HD = 4, 1024, 768, 12, 64
HID = 4 * C
EPS = 1e-6
SCALE = HD ** -0.5
NT = T // 128      # 8 token tiles
KC = C // 128      # 6 feature chunks
NH = T // 512      # 2 psum halves
NHT = HID // 128   # 24 hidden tiles
NP = H // 2        # 6 head pairs
N_CORES = 8
GROUPS = [[0, 1], [2, 3], [4, 5], [6, 7]]

_CACHE = {}


def _emit(nc):
    dp = nc.declare_dram_parameter
    tok_d = dp("tok", [T, C], F32R, isOutput=False)
    ident_d = dp("ident", [128, 128], F32R, isOutput=False)
    onesf_d = dp("onesf", [128, 128], F32R, isOutput=False)
    onesb_d = dp("onesb", [128, 128], BF16, isOutput=False)
    wqk_d = dp("wqk", [C, 2 * C], BF16, isOutput=False)
    bqk_d = dp("bqk", [2 * C], F32, isOutput=False)
    wvs_d = dp("wvs", [C, C], BF16, isOutput=False)
    wproj_d = dp("wproj", [C, C], BF16, isOutput=False)
    bproj_d = dp("bproj", [C], F32, isOutput=False)
    wab_d = dp("wab", [C, 2 * C], BF16, isOutput=False)
    bab_d = dp("bab", [2 * C], F32, isOutput=False)
    wvc_d = dp("wvc", [C, C], BF16, isOutput=False)
    wcp_d = dp("wcp", [C, C], BF16, isOutput=False)
    bcp_d = dp("bcp", [C], F32, isOutput=False)
    wm1_d = dp("wm1", [C, HID], BF16, isOutput=False)
    bm1_d = dp("bm1", [HID], F32, isOutput=False)
    wm2_d = dp("wm2", [HID, C], BF16, isOutput=False)
    bm2_d = dp("bm2", [C], F32, isOutput=False)
    out_d = dp("out_tok", [T, C], F32, isOutput=True)

    with tile.TileContext(nc) as tc, nc.allow_low_precision(reason="bf16 kernel"):
        with (
            tc.tile_pool(name="pP", bufs=1) as pP,
            tc.tile_pool(name="pB", bufs=4) as pB,
            tc.tile_pool(name="pROW", bufs=4) as pROW,
            tc.tile_pool(name="pROWF", bufs=2) as pROWF,
            tc.tile_pool(name="pDEN", bufs=2) as pDEN,
            tc.tile_pool(name="ps", bufs=3, space="PSUM") as psp,
            tc.tile_pool(name="pst", bufs=2, space="PSUM") as pstp,
            tc.tile_pool(name="dram", bufs=1, space="DRAM") as dram,
        ):
            ident = pP.tile([128, 128], F32R, tag="ident")
            onesm = pP.tile([128, 128], F32R, tag="onesm")
            onesb = pP.tile([128, 128], BF16, tag="onesb")
            nc.sync.dma_start(out=ident[:], in_=ident_d[:])
            nc.sync.dma_start(out=onesm[:], in_=onesf_d[:])
            nc.sync.dma_start(out=onesb[:], in_=onesb_d[:])
            eps_t = pP.tile([1, 1], F32, tag="epst")
            nc.vector.memset(eps_t[:], EPS)

            def bias_tile(b_dram, off):
                bt = pB.tile([128, 1], F32, tag="BIAS", name="bt")
                nc.sync.dma_start(
                    out=bt[:],
                    in_=b_dram[off:off + 128].rearrange("(p o) -> p o", o=1),
                )
                return bt

            def load_slabs(w_dram, width, n, pool, tag):
                slabs = []
                for kc in range(n):
                    s = pool.tile([128, width], BF16, tag=tag, name=f"{tag}{kc}")
                    nc.sync.dma_start(
                        out=s[:], in_=w_dram[kc * 128:(kc + 1) * 128, :])
                    slabs.append(s)
                return slabs

            # ---- residual stream tiles (persistent, f32 feature-major) ----
            xT = [pP.tile([128, T], F32R, tag=f"XT{i}", name=f"xT{i}")
                  for i in range(KC)]

            # ---- phase 0: load tokens, transpose to feature-major ----
            with tc.tile_pool(name="pTOK", bufs=8) as pTOK:
                tok_sb = []
                for tt in range(NT):
                    t_ = pTOK.tile([128, C], F32R, tag="TOK", name="tok_t")
                    nc.sync.dma_start(out=t_[:],
                                      in_=tok_d[tt * 128:(tt + 1) * 128, :])
                    tok_sb.append(t_)
                for cc in range(KC):
                    for tt in range(NT):
                        pt = pstp.tile([128, 512], F32R, tag="TP", name="pt")
                        nc.tensor.transpose(
                            pt[:, 0:128], tok_sb[tt][:, cc * 128:(cc + 1) * 128],
                            ident[:],
                        )
                        nc.vector.tensor_copy(
                            xT[cc][:, tt * 128:(tt + 1) * 128],
                            pt[:, 0:128],
                        )

            # ---- helpers ----
            def layer_norm(pXH, pSQ):
                """LN of xT -> bf16 normalized tiles."""
                ps_mu = psp.tile([128, T], F32, tag="PS", name="psmu")
                ps_sq = psp.tile([128, T], F32, tag="PS", name="pssq")
                for kc in range(KC):
                    sqt = pSQ.tile([128, T], F32R, tag="SQT", name="sqt")
                    nc.scalar.activation(sqt[:], xT[kc][:].bitcast(F32),
                                         AF.Square)
                    for hf in range(NH):
                        sl = slice(hf * 512, (hf + 1) * 512)
                        nc.tensor.matmul(
                            ps_mu[0:1, sl], onesm[:, 0:1], xT[kc][:, sl],
                            start=(kc == 0), stop=(kc == KC - 1),
                        )
                        nc.tensor.matmul(
                            ps_sq[0:1, sl], onesm[:, 0:1], sqt[:, sl],
                            start=(kc == 0), stop=(kc == KC - 1),
                        )
                mu_b = pROW.tile([1, T], BF16, tag="ROW", name="mu")
                nc.vector.tensor_scalar(
                    mu_b[:], ps_mu[0:1, :], 1.0 / C, None, op0=OP.mult)
                mv = pROWF.tile([1, T], F32, tag="ROWF", name="mv")
                nc.vector.tensor_tensor(mv[:], mu_b[:], mu_b[:], OP.mult)
                nc.vector.scalar_tensor_tensor(
                    out=mv[:], in0=ps_sq[0:1, :], scalar=1.0 / C, in1=mv[:],
                    op0=OP.mult, op1=OP.subtract,
                )
                nc.scalar.activation(mv[:], mv[:], AF.Ln, bias=eps_t[:])
                rstd_b = pROW.tile([1, T], BF16, tag="ROW", name="rstd")
                nc.scalar.activation(rstd_b[:], mv[:], AF.Exp, scale=-0.5)
                mr_b = pROW.tile([1, T], BF16, tag="ROW", name="mr")
                nc.vector.tensor_tensor(mr_b[:], mu_b[:], rstd_b[:], OP.mult)

                ps_rb = psp.tile([128, T], F32, tag="PS", name="psrb")
                ps_mb = psp.tile([128, T], F32, tag="PS", name="psmb")
                for hf in range(NH):
                    sl = slice(hf * 512, (hf + 1) * 512)
                    nc.tensor.matmul(ps_rb[:, sl], onesb[0:1, :],
                                     rstd_b[0:1, sl], start=True, stop=True)
                    nc.tensor.matmul(ps_mb[:, sl], onesb[0:1, :],
                                     mr_b[0:1, sl], start=True, stop=True)
                rbs = pSQ.tile([128, T], BF16, tag="RBS", name="rbs")
                mbs = pSQ.tile([128, T], BF16, tag="RBS", name="mbs")
                nc.vector.tensor_copy(rbs[:], ps_rb[:])
                nc.vector.tensor_copy(mbs[:], ps_mb[:])
                out = []
                for kc in range(KC):
                    xh = pXH.tile([128, T], BF16, tag="XH", name="xh")
                    nc.vector.tensor_tensor(xh[:], xT[kc][:].bitcast(F32),
                                            rbs[:], OP.mult)
                    nc.vector.tensor_tensor(xh[:], xh[:], mbs[:], OP.subtract)
                    out.append(xh)
                return out

            def proj_fm(xh, slabs, b_dram, col_off, pool, tag):
                """Feature-major projection: out[ot] = (W.T @ xh) + b, bf16."""
                outs = []
                for ot in range(KC):
                    pp = psp.tile([128, T], F32, tag="PS", name="pp")
                    for hf in range(NH):
                        sl = slice(hf * 512, (hf + 1) * 512)
                        for kc in range(KC):
                            nc.tensor.matmul(
                                pp[:, sl],
                                slabs[kc][:, col_off + ot * 128:
                                          col_off + (ot + 1) * 128],
                                xh[kc][:, sl],
                                start=(kc == 0), stop=(kc == KC - 1),
                            )
                    bt = bias_tile(b_dram, col_off + ot * 128)
                    o = pool.tile([128, T], BF16, tag=tag, name="o")
                    nc.scalar.activation(o[:], pp[:], AF.Identity,
                                         bias=bt[:], scale=1.0)
                    outs.append(o)
                return outs

            def proj_vaug(xh, slabs, pVA):
                """Token-major V with ones column appended per head (bf16)."""
                outs = []
                for mt in range(NT):
                    pp = psp.tile([128, T], F32, tag="PS", name="ppv")
                    for ck in range(2):  # 768 = 512 + 256 moving chunks
                        sl = slice(ck * 512, min((ck + 1) * 512, C))
                        for kc in range(KC):
                            nc.tensor.matmul(
                                pp[:, sl],
                                xh[kc][:, mt * 128:(mt + 1) * 128],
                                slabs[kc][:, sl],
                                start=(kc == 0), stop=(kc == KC - 1),
                            )
                    va = pVA.tile([128, 780], BF16, tag="VA", name="va")
                    va3 = va[:].rearrange("p (h e) -> p h e", e=65)
                    nc.scalar.copy(
                        va3[:, :, 0:64],
                        pp[:, 0:C].rearrange("p (h e) -> p h e", e=64),
                    )
                    nc.vector.memset(va3[:, :, 64:65], 1.0)
                    outs.append(va)
                return outs

            def attention(qT, kT, va, pOT, pPT):
                """Heads packed in pairs: head 2c rows 0:64, 2c+1 rows 64:128
                of tile c. QK^T row-packed (concurrent on PE row groups);
                denominators batched into one reciprocal."""
                oT = [pOT.tile([128, T], BF16, tag="OT", name=f"oT{i}")
                      for i in range(KC)]
                for c in range(NP):
                    pts = ([], [])
                    for mt in range(NT):
                        for half in range(2):
                            ro = half * 64
                            psS = psp.tile([128, T], F32, tag="PS", name="psS")
                            for hf in range(NH):
                                sl = slice(hf * 512, (hf + 1) * 512)
                                nc.tensor.matmul(
                                    psS[:, sl],
                                    kT[c][ro:ro + HD, mt * 128:(mt + 1) * 128],
                                    qT[c][ro:ro + HD, sl],
                                    start=True, stop=True,
                                )
                            pt = pPT.tile([128, T], BF16, tag="PT", name="pt")
                            nc.scalar.activation(pt[:], psS[:], AF.Exp)
                            pts[half].append(pt)
                    rcpb = []
                    for half in range(2):
                        h = 2 * c + half
                        ro = half * 64
                        drow = pDEN.tile([1, T], F32, tag="DR", name="drow")
                        for hf in range(NH):
                            sl = slice(hf * 512, (hf + 1) * 512)
                            psO = pstp.tile([128, 512], F32, tag="TP",
                                            name="psO")
                            for mt in range(NT):
                                nc.tensor.matmul(
                                    psO[0:65, :],
                                    va[mt][:, h * 65:(h + 1) * 65],
                                    pts[half][mt][:, sl],
                                    start=(mt == 0), stop=(mt == NT - 1),
                                )
                            nc.vector.tensor_copy(oT[c][ro:ro + 64, sl],
                                                  psO[0:64, :])
                            nc.vector.tensor_copy(drow[0:1, sl],
                                                  psO[64:65, :])
                        rcpf = pDEN.tile([1, T], F32, tag="RCF", name="rcpf")
                        nc.vector.reciprocal_approx_fast(rcpf[:], drow[:])
                        rb = pROW.tile([1, T], BF16, tag="ROW", name="rb")
                        nc.vector.tensor_copy(rb[:], rcpf[:])
                        rcpb.append(rb)
                    ps_rb = psp.tile([128, T], F32, tag="PS", name="psrcp")
                    for hf in range(NH):
                        sl = slice(hf * 512, (hf + 1) * 512)
                        nc.tensor.matmul(
                            ps_rb[0:64, sl], onesb[0:1, 0:64],
                            rcpb[0][0:1, sl], start=True, stop=True)
                        nc.tensor.matmul(
                            ps_rb[64:128, sl], onesb[0:1, 0:64],
                            rcpb[1][0:1, sl], start=True, stop=True)
                    for hf in range(NH):
                        sl = slice(hf * 512, (hf + 1) * 512)
                        nc.vector.tensor_tensor(
                            oT[c][:, sl], oT[c][:, sl], ps_rb[:, sl], OP.mult)
                return oT

            def proj_residual(oT, slabs, b_dram):
                for ot in range(KC):
                    pp = psp.tile([128, T], F32, tag="PS", name="ppr")
                    for hf in range(NH):
                        sl = slice(hf * 512, (hf + 1) * 512)
                        for kc in range(KC):
                            nc.tensor.matmul(
                                pp[:, sl],
                                slabs[kc][:, ot * 128:(ot + 1) * 128],
                                oT[kc][:, sl],
                                start=(kc == 0), stop=(kc == KC - 1),
                            )
                    bt = bias_tile(b_dram, ot * 128)
                    nc.vector.scalar_tensor_tensor(
                        out=xT[ot][:], in0=pp[:], scalar=bt[:],
                        in1=xT[ot][:].bitcast(F32), op0=OP.add, op1=OP.add,
                    )

            # ================= self-attention =================
            with (
                tc.tile_pool(name="pQK_s", bufs=6) as pQK,
                tc.tile_pool(name="pVA_s", bufs=9) as pVA,
                tc.tile_pool(name="pOT_s", bufs=6) as pOT,
                tc.tile_pool(name="pPT_s", bufs=20) as pPT,
            ):
                with (
                    tc.tile_pool(name="pXH_s", bufs=6) as pXH,
                    tc.tile_pool(name="pSQ_s", bufs=2) as pSQ,
                    tc.tile_pool(name="pWS_s", bufs=6) as pWS,
                ):
                    xh1 = layer_norm(pXH, pSQ)
                    wqk_s = load_slabs(wqk_d, 2 * C, KC, pWS, "WQK")
                    qT = proj_fm(xh1, wqk_s, bqk_d, 0, pQK, "QT")
                    kT = proj_fm(xh1, wqk_s, bqk_d, C, pQK, "KT")
                    wvs_s = load_slabs(wvs_d, C, KC, pWS, "WVS")
                    va_s = proj_vaug(xh1, wvs_s, pVA)
                oT = attention(qT, kT, va_s, pOT, pPT)
                with tc.tile_pool(name="pWP_s", bufs=6) as pWP:
                    wp_s = load_slabs(wproj_d, C, KC, pWP, "WP")
                    proj_residual(oT, wp_s, bproj_d)

            # ================= cross-attention ================
            with (
                tc.tile_pool(name="pQK_c", bufs=6) as pQK,
                tc.tile_pool(name="pVA_c", bufs=9) as pVA,
                tc.tile_pool(name="pOT_c", bufs=6) as pOT,
                tc.tile_pool(name="pPT_c", bufs=20) as pPT,
                tc.tile_pool(name="pRS", bufs=2) as pRS,
            ):
                with (
                    tc.tile_pool(name="pXH_c", bufs=6) as pXH,
                    tc.tile_pool(name="pSQ_c", bufs=2) as pSQ,
                    tc.tile_pool(name="pWB_c", bufs=6) as pWB,
                ):
                    xhc = layer_norm(pXH, pSQ)
                    wab_s = load_slabs(wab_d, 2 * C, KC, pWB, "WAB")
                    bT = proj_fm(xhc, wab_s, bab_d, C, pQK, "KT")
                    with tc.tile_pool(name="pWV_c", bufs=6) as pWV:
                        wvc_s = load_slabs(wvc_d, C, KC, pWV, "WVC")
                        va_c = proj_vaug(xhc, wvc_s, pVA)

                    ccb_in = dram.tile([C, T], BF16, tag="ccb_in")
                    ccb_out = dram.tile([C, T], BF16, tag="ccb_out")
                    ccv_in = dram.tile([T, 780], BF16, tag="ccv_in")
                    ccv_out = dram.tile([T, 780], BF16, tag="ccv_out")
                    for cc in range(KC):
                        nc.sync.dma_start(
                            out=ccb_in[cc * 128:(cc + 1) * 128, :],
                            in_=bT[cc][:])
                    for mt in range(NT):
                        nc.sync.dma_start(
                            out=ccv_in[mt * 128:(mt + 1) * 128, :],
                            in_=va_c[mt][:])
                    nc.gpsimd.collective_compute(
                        "AllReduce", OP.add, replica_groups=GROUPS,
                        ins=[ccb_in.opt()], outs=[ccb_out.opt()],
                    )
                    nc.gpsimd.collective_compute(
                        "AllReduce", OP.add, replica_groups=GROUPS,
                        ins=[ccv_in.opt()], outs=[ccv_out.opt()],
                    )
                    # overlap the collective with the local query projection
                    aT = proj_fm(xhc, wab_s, bab_d, 0, pQK, "QT")

                # partner = sum - mine, in place
                for cc in range(KC):
                    st = pRS.tile([128, T], BF16, tag="RS", name="st")
                    nc.sync.dma_start(out=st[:],
                                      in_=ccb_out[cc * 128:(cc + 1) * 128, :])
                    nc.vector.tensor_sub(bT[cc][:], st[:], bT[cc][:])
                for mt in range(NT):
                    sv = pRS.tile([128, 780], BF16, tag="RSV", name="sv")
                    nc.sync.dma_start(out=sv[:],
                                      in_=ccv_out[mt * 128:(mt + 1) * 128, :])
                    nc.vector.tensor_sub(va_c[mt][:], sv[:], va_c[mt][:])

                oTc = attention(aT, bT, va_c, pOT, pPT)
                with tc.tile_pool(name="pWP_c", bufs=6) as pWP:
                    wcp_s = load_slabs(wcp_d, C, KC, pWP, "WCP")
                    proj_residual(oTc, wcp_s, bcp_d)

            # ================= MLP =================
            with (
                tc.tile_pool(name="pXH_m", bufs=6) as pXH,
                tc.tile_pool(name="pSQ_m", bufs=2) as pSQ,
                tc.tile_pool(name="pHT", bufs=24) as pHT,
                tc.tile_pool(name="pW1_m", bufs=6) as pW1,
                tc.tile_pool(name="pW2_m", bufs=24) as pW2,
            ):
                xh2 = layer_norm(pXH, pSQ)
                w1_s = load_slabs(wm1_d, HID, KC, pW1, "W1")
                w2_s = load_slabs(wm2_d, C, NHT, pW2, "W2")
                hT = []
                for ht in range(NHT):
                    pp = psp.tile([128, T], F32, tag="PS", name="pph")
                    for hf in range(NH):
                        sl = slice(hf * 512, (hf + 1) * 512)
                        for kc in range(KC):
                            nc.tensor.matmul(
                                pp[:, sl],
                                w1_s[kc][:, ht * 128:(ht + 1) * 128],
                                xh2[kc][:, sl],
                                start=(kc == 0), stop=(kc == KC - 1),
                            )
                    bt = bias_tile(bm1_d, ht * 128)
                    h_sb = pHT.tile([128, T], BF16, tag="HT", name="h_sb")
                    nc.scalar.activation(h_sb[:], pp[:], AF.Gelu,
                                         bias=bt[:], scale=1.0)
                    hT.append(h_sb)
                for ot in range(KC):
                    pp = psp.tile([128, T], F32, tag="PS", name="pp2")
                    for hf in range(NH):
                        sl = slice(hf * 512, (hf + 1) * 512)
                        for ht in range(NHT):
                            nc.tensor.matmul(
                                pp[:, sl],
                                w2_s[ht][:, ot * 128:(ot + 1) * 128],
                                hT[ht][:, sl],
                                start=(ht == 0), stop=(ht == NHT - 1),
                            )
                    bt = bias_tile(bm2_d, ot * 128)
                    nc.vector.scalar_tensor_tensor(
                        out=xT[ot][:], in0=pp[:], scalar=bt[:],
                        in1=xT[ot][:].bitcast(F32), op0=OP.add, op1=OP.add,
                    )

            # ---- output transpose + DMA ----
            with tc.tile_pool(name="pOUT", bufs=2) as pOUT:
                for tt in range(NT):
                    ot_sb = pOUT.tile([128, C], F32, tag="OUTT", name="ot_sb")
                    for cc in range(KC):
                        pt = pstp.tile([128, 512], F32R, tag="TP", name="pt")
                        nc.tensor.transpose(
                            pt[:, 0:128],
                            xT[cc][:, tt * 128:(tt + 1) * 128],
                            ident[:],
                        )
                        nc.vector.tensor_copy(
                            ot_sb[:, cc * 128:(cc + 1) * 128],
                            pt[:, 0:128].bitcast(F32),
                        )
                    nc.sync.dma_start(out=out_d[tt * 128:(tt + 1) * 128, :],
                                      in_=ot_sb[:])

    nc.compile()
    return nc


def _build():
    if "nc" not in _CACHE:
        nc = bacc.Bacc("TRN2", target_bir_lowering=False)
        _CACHE["nc"] = _emit(nc)
    return _CACHE["nc"]


def _fold_ln(w, ln_w, ln_b):
    """w [out, in]; returns (w', b') with LN scale/bias folded in."""
    w = np.asarray(w, np.float64)
    wf = w * np.asarray(ln_w, np.float64)[None, :]
    bf = w @ np.asarray(ln_b, np.float64)
    return wf, bf


def _core_inputs(s, tok, p):
    sfx = "" if s == 0 else "s"
    wqkv, bqkv = _fold_ln(p["w_qkv" + ("" if s == 0 else "_s")],
                          p[f"ln1{sfx}_w"], p[f"ln1{sfx}_b"])
    wqkv = wqkv.copy()
    wqkv[:C] *= SCALE
    bqkv = bqkv.copy()
    bqkv[:C] *= SCALE
    wproj = np.asarray(p["w_proj" + ("" if s == 0 else "_s")], np.float64)
    bproj = np.asarray(p["b_proj" + ("" if s == 0 else "_s")], np.float64) \
        + wproj @ bqkv[2 * C:]
    lncw = p["lnc_w" if s == 0 else "lncs_w"]
    lncb = p["lnc_b" if s == 0 else "lncs_b"]
    wqk, bqk_ = _fold_ln(p["w_qk" if s == 0 else "w_qk_src"], lncw, lncb)
    wqk3 = wqk.reshape(H, 2 * HD, C)
    bqk3 = bqk_.reshape(H, 2 * HD)
    if s == 0:
        A, Ab = wqk3[:, :HD] * SCALE, bqk3[:, :HD] * SCALE
        Bm, Bb = wqk3[:, HD:] * SCALE, bqk3[:, HD:] * SCALE
    else:
        A, Ab = wqk3[:, HD:], bqk3[:, HD:]
        Bm, Bb = wqk3[:, :HD], bqk3[:, :HD]
    wab = np.concatenate([A.reshape(C, C), Bm.reshape(C, C)], axis=0)
    bab = np.concatenate([Ab.reshape(C), Bb.reshape(C)], axis=0)
    wvc, bvc = _fold_ln(p["w_v" if s == 0 else "w_v_src"], lncw, lncb)
    wcp = np.asarray(p["w_cp" if s == 0 else "w_cp_src"], np.float64)
    bcp = np.asarray(p["b_cp" if s == 0 else "b_cp_src"], np.float64) + wcp @ bvc
    wm1, bm1 = _fold_ln(p[f"mlp1{sfx}_w"], p[f"ln2{sfx}_w"], p[f"ln2{sfx}_b"])
    bm1 = bm1 + np.asarray(p[f"mlp1{sfx}_b"], np.float64)
    wm2 = np.asarray(p[f"mlp2{sfx}_w"], np.float64)
    bm2 = np.asarray(p[f"mlp2{sfx}_b"], np.float64)

    f32 = lambda a: np.ascontiguousarray(a, np.float32)
    bf = lambda a: np.ascontiguousarray(
        np.asarray(a, np.float32)).astype(ml_dtypes.bfloat16)
    return {
        "tok": f32(tok),
        "ident": f32(np.eye(128)),
        "onesf": f32(np.ones((128, 128))),
        "onesb": bf(np.ones((128, 128))),
        "wqk": bf(wqkv[:2 * C].T),
        "bqk": f32(bqkv[:2 * C]),
        "wvs": bf(wqkv[2 * C:].T),
        "wproj": bf(wproj.T),
        "bproj": f32(bproj),
        "wab": bf(wab.T),
        "bab": f32(bab),
        "wvc": bf(wvc.T),
        "wcp": bf(wcp.T),
        "bcp": f32(bcp),
        "wm1": bf(wm1.T),
        "bm1": f32(bm1),
        "wm2": bf(wm2.T),
        "bm2": f32(bm2),
    }


def make_in_maps(inputs):
    x = np.asarray(inputs["x"])
    src = np.asarray(inputs["src"])
    maps = []
    for b in range(B):
        for s in range(2):
            maps.append(_core_inputs(s, x[b] if s == 0 else src[b], inputs))
    return maps


def kernel(**inputs):
    nc = _build()
    in_maps = make_in_maps(inputs)
    res = run_bass_kernel_spmd(nc, in_maps, list(range(N_CORES)))
    x_out = np.stack([res.results[2 * b]["out_tok"] for b in range(B)])
    src_out = np.stack([res.results[2 * b + 1]["out_tok"] for b in range(B)])
    return (x_out.astype(np.float32), src_out.astype(np.float32))
